# revision 1
# baseline (speedup 1.0000x reference)
"""Trainium2 Bass kernel for the 2-layer ReLU-RNN discriminator.

  B=64, T=512, I=256, H=512, O=1
  layer l: h_t = relu(x_t @ W_ih^T + b_ih + b_hh + h_{t-1} @ W_hh^T)
  out = sigmoid(h1 @ W_fc^T + b_fc)

Sharding: data-parallel over batch, 8 sequences per NeuronCore.

Per-core design (all activations/weights bf16, PSUM accumulation fp32):
- hidden state kept TRANSPOSED: column t*8+b of a [H=512(4x128 part-chunks), .]
  buffer holds h_t for local sample b. The recurrence then needs no
  transposes: stationary = W_hh^T tile [128,128], moving = h chunk [128,8],
  psum out [128(h_out chunk), 8] in the same transposed layout.
- x^T is prepared host-side (layout prep of the sharded input).
- xw = x @ W_ih^T + b precomputed as chunked GEMMs (64 timesteps/chunk).
- L0 and L1 recurrences are software-interleaved with a CH-step delay so the
  PE always has an independent matmul stream while the other layer's
  add+relu (both on DVE - keeping ACT out of the per-step chain measured ~3x
  faster) completes; otherwise the chain PE->DVE->PE stalls the PE each step.
- k-outer matmul order + per-m epilogues on 4 single-bank PSUM tiles let the
  next step's k=0 matmuls gate only on the m=0 epilogue.
"""

import numpy as np
import ml_dtypes

import concourse.bass as bass
import concourse.mybir as mybir
from concourse.tile import TileContext
from concourse.bass_utils import run_bass_kernel_spmd
from concourse.alu_op_type import AluOpType

BF16 = ml_dtypes.bfloat16
NCORES = 8
B, T, I, H, O = 64, 512, 256, 512, 1
BL = B // NCORES          # sequences per core
import os
CH = int(os.environ.get('K_CH', '32'))  # timesteps per chunk
NCH = T // CH             # chunks
KI = I // 128             # k-chunks of the input dim
KH = H // 128             # k-chunks / m-chunks of the hidden dim
W = BL * KH               # packed step width: 4 m-chunks x 8 samples = 32

_ctr = [0]


def _split_multi_waits(nc):
    """This container's walrus build rejects >1 sync-wait per instruction
    ("Too many sync wait commands"). Rewrite any instruction with N>1 waits
    into N-1 preceding single-wait NOPs on the same engine."""
    n_split = 0
    for f in nc.m.functions:
        for bb in f.blocks:
            out = []
            changed = False
            for inst in bb.instructions:
                si = inst.sync_info
                waits = list(si.on_wait) if si is not None and si.on_wait else []
                if len(waits) > 1:
                    changed = True
                    n_split += 1
                    for w in waits[:-1]:
                        _ctr[0] += 1
                        nop = mybir.InstNoOp(
                            name=f"waitnop-{_ctr[0]}", ins=[], outs=[]
                        )
                        nop.engine = inst.engine
                        nop.sync_info = mybir.SyncInfo(on_wait=[w], on_update=[])
                        out.append(nop)
                    inst.sync_info = mybir.SyncInfo(
                        on_wait=[waits[-1]],
                        on_update=list(si.on_update) if si.on_update else [],
                    )
                out.append(inst)
            if changed:
                bb.instructions = out
    return n_split


def build_nc(n_steps=T, split_waits=True, debug_dumps=False, delay=CH):
    nsc = n_steps // CH  # number of chunks actually used
    nc = bass.Bass("TRN2", num_devices=NCORES)
    f32, bf = mybir.dt.float32, mybir.dt.bfloat16

    xt_d = nc.dram_tensor("xt", [I, n_steps * BL], bf, kind="ExternalInput")
    w0i_d = nc.dram_tensor("w0i", [I, H], bf, kind="ExternalInput")
    w0h_d = nc.dram_tensor("w0h", [H, H], bf, kind="ExternalInput")
    w1i_d = nc.dram_tensor("w1i", [H, H], bf, kind="ExternalInput")
    w1h_d = nc.dram_tensor("w1h", [H, H], bf, kind="ExternalInput")
    MF = 8
    wfc_d = nc.dram_tensor("wfc", [128, KH * MF], bf, kind="ExternalInput")
    b0_d = nc.dram_tensor("b0", [128, KH], f32, kind="ExternalInput")
    b1_d = nc.dram_tensor("b1", [128, KH], f32, kind="ExternalInput")
    bfc_d = nc.dram_tensor("bfc", [1, 1], f32, kind="ExternalInput")
    y_d = nc.dram_tensor("y", [nsc, CH * BL], f32, kind="ExternalOutput")
    bf_np = mybir.dt.bfloat16
    if debug_dumps:
        dbg = {
            "dxw0": nc.dram_tensor("dxw0", [128, CH * W], bf_np, kind="ExternalOutput"),
            "dh0": nc.dram_tensor("dh0", [128, KH * CH * BL], bf_np, kind="ExternalOutput"),
            "dxw1": nc.dram_tensor("dxw1", [128, CH * W], bf_np, kind="ExternalOutput"),
            "dh1": nc.dram_tensor("dh1", [128, KH * CH * BL], bf_np, kind="ExternalOutput"),
        }

    with TileContext(nc) as tc:
        with (
            tc.tile_pool(name="xt", bufs=KI) as p_xt,
            tc.tile_pool(name="wts", bufs=6) as p_w,
            tc.tile_pool(name="h0", bufs=nsc) as p_h0,
            tc.tile_pool(name="h1", bufs=nsc) as p_h1,
            tc.tile_pool(name="xw0", bufs=nsc) as p_xw0,
            tc.tile_pool(name="xw1", bufs=nsc) as p_xw1,
            tc.tile_pool(name="z", bufs=16) as p_z,
            tc.tile_pool(name="fco", bufs=2) as p_fco,
            tc.tile_pool(name="psr", bufs=1, space="PSUM") as p_psr,
            tc.tile_pool(name="psg", bufs=2, space="PSUM") as p_psg,
            tc.tile_pool(name="psfc", bufs=2, space="PSUM") as p_psfc,
        ):
            # ---- load inputs to SBUF ----
            xt_sb = []
            for k in range(KI):
                t_ = p_xt.tile([128, n_steps * BL], bf, tag="xt", name=f"xtsb{k}")
                nc.sync.dma_start(t_[:], xt_d[k * 128:(k + 1) * 128, :])
                xt_sb.append(t_)

            def load_w(dram, kchunks):
                t_ = p_w.tile([128, kchunks * H], bf, tag="w", name=f"w{_ctr[0]}") ; _ctr[0] += 1
                for k in range(kchunks):
                    nc.sync.dma_start(
                        t_[:, k * H:(k + 1) * H], dram[k * 128:(k + 1) * 128, :]
                    )
                return t_

            w0i_sb = load_w(w0i_d, KI)
            w0h_sb = load_w(w0h_d, KH)
            w1i_sb = load_w(w1i_d, KH)
            w1h_sb = load_w(w1h_d, KH)
            wfc_sb = p_w.tile([128, KH * MF], bf, tag="small")
            nc.sync.dma_start(wfc_sb[:], wfc_d[:])
            b0_sb = p_w.tile([128, KH], f32, tag="small")
            nc.sync.dma_start(b0_sb[:], b0_d[:])
            b1_sb = p_w.tile([128, KH], f32, tag="small")
            nc.sync.dma_start(b1_sb[:], b1_d[:])
            bfc_sb = p_w.tile([1, 1], f32, tag="small")
            nc.sync.dma_start(bfc_sb[:], bfc_d[:])

            # persistent chunk tiles
            h0c = [p_h0.tile([128, KH * CH * BL], bf, tag="h0", name=f"h0c{i}")
                   for i in range(nsc)]
            h1c = [p_h1.tile([128, KH * CH * BL], bf, tag="h1", name=f"h1c{i}")
                   for i in range(nsc)]
            xw0c = [p_xw0.tile([128, CH * W], bf, tag="xw0", name=f"xw0c{i}")
                    for i in range(nsc)]
            xw1c = [p_xw1.tile([128, CH * W], bf, tag="xw1", name=f"xw1c{i}")
                    for i in range(nsc)]

            def r3_h(tile):   # [128, KH, CH*BL]
                return tile[:].rearrange("p (k x) -> p k x", k=KH)

            def r3_xw(tile):  # [128, CH, W]
                return tile[:].rearrange("p (t w) -> p t w", w=W)

            # ---- input GEMM for layer 0: xw0 = x @ W_ih0^T + b0 ----
            def gemm0(c):
                for m in range(KH):
                    ps = p_psg.tile([128, CH * BL], f32, tag="psg", name=f"psg{_ctr[0]}"); _ctr[0] += 1
                    for k in range(KI):
                        nc.tensor.matmul(
                            ps[:],
                            w0i_sb[:, k * H + m * 128: k * H + (m + 1) * 128],
                            xt_sb[k][:, c * CH * BL:(c + 1) * CH * BL],
                            start=(k == 0),
                            stop=(k == KI - 1),
                        )
                    nc.scalar.activation(
                        r3_xw(xw0c[c])[:, :, m * BL:(m + 1) * BL],
                        ps[:].rearrange("p (t b) -> p t b", b=BL),
                        mybir.ActivationFunctionType.Identity,
                        bias=b0_sb[:, m:m + 1],
                    )

            # ---- input GEMM for layer 1 (consumes finished h0 chunk) ----
            def gemm1(c):
                for m in range(KH):
                    ps = p_psg.tile([128, CH * BL], f32, tag="psg", name=f"psg{_ctr[0]}"); _ctr[0] += 1
                    for k in range(KH):
                        nc.tensor.matmul(
                            ps[:],
                            w1i_sb[:, k * H + m * 128: k * H + (m + 1) * 128],
                            h0c[c][:, k * CH * BL:(k + 1) * CH * BL],
                            start=(k == 0),
                            stop=(k == KH - 1),
                        )
                    nc.scalar.activation(
                        r3_xw(xw1c[c])[:, :, m * BL:(m + 1) * BL],
                        ps[:].rearrange("p (t b) -> p t b", b=BL),
                        mybir.ActivationFunctionType.Identity,
                        bias=b1_sb[:, m:m + 1],
                    )

            # ---- one recurrence step (shared by both layers) ----
            # 4 single-bank psum tiles, one per output m-chunk, shared by both
            # layers (their groups alternate in the PE stream). k-outer MM
            # order + per-m epilogue: relu for m completes while later k
            # groups still run, so the next step's k=0 matmuls (which need
            # only h chunk 0) are gated by an epilogue that finished early.
            def rec_step(t, whh_sb, xwc, hc, z_tag):
                c, r = divmod(t, CH)
                if t == 0:
                    # h_{-1} = 0: h_0 = relu(xw_0)
                    for m in range(KH):
                        nc.vector.tensor_scalar_max(
                            hc[0][:, m * CH * BL: m * CH * BL + BL],
                            xwc[0][:, m * BL:(m + 1) * BL], 0.0)
                    return
                pc, pr = divmod(t - 1, CH)
                ps0 = p_psr.tile([128, BL], f32, tag=z_tag + "p0",
                                 name=f"p0{z_tag}{t}")
                ps123 = p_psr.tile([128, (KH - 1) * BL], f32,
                                   tag=z_tag + "p123",
                                   name=f"p123{z_tag}{t}")

                def psl(m):
                    if m == 0:
                        return ps0[:]
                    return ps123[:, (m - 1) * BL:m * BL]

                for k in range(KH):
                    rhs = hc[pc][:, k * CH * BL + pr * BL:
                                 k * CH * BL + (pr + 1) * BL]
                    for m in range(KH):
                        nc.tensor.matmul(
                            psl(m),
                            whh_sb[:, k * H + m * 128: k * H + (m + 1) * 128],
                            rhs,
                            start=(k == 0),
                            stop=(k == KH - 1),
                        )
                        if k == KH - 1:
                            hdst = hc[c][:, m * CH * BL + r * BL:
                                         m * CH * BL + (r + 1) * BL]
                            xsl = xwc[c][:, r * W + m * BL:
                                         r * W + (m + 1) * BL]
                            if m == 0:
                                z = p_z.tile([128, BL], bf, tag=z_tag,
                                             name=f"z{z_tag}{t}m{m}")
                                nc.vector.tensor_tensor(
                                    z[:], ps0[:], xsl, AluOpType.add)
                                nc.vector.tensor_scalar_max(hdst, z[:], 0.0)
                            elif m == KH - 1:
                                z3 = p_z.tile([128, (KH - 1) * BL], bf,
                                              tag=z_tag + "w",
                                              name=f"zw{z_tag}{t}")
                                x3 = xwc[c][:, r * W + BL: r * W + KH * BL]
                                nc.vector.tensor_tensor(
                                    z3[:], ps123[:], x3, AluOpType.add)
                                nc.vector.tensor_scalar_max(
                                    r3_h(hc[c])[:, 1:KH,
                                                r * BL:(r + 1) * BL],
                                    z3[:].rearrange(
                                        "p (q b) -> p q b", b=BL),
                                    0.0)

            # ---- final FC + sigmoid for a finished h1 chunk ----
            def fc(c):
                ps = p_psfc.tile([MF, CH * BL], f32, tag="psfc", name=f"psfc{c}")
                for k in range(KH):
                    nc.tensor.matmul(
                        ps[:],
                        wfc_sb[:, k * MF:(k + 1) * MF],
                        h1c[c][:, k * CH * BL:(k + 1) * CH * BL],
                        start=(k == 0),
                        stop=(k == KH - 1),
                    )
                o = p_fco.tile([1, CH * BL], f32, tag="fco", name=f"fco{c}")
                nc.scalar.activation(
                    o[:], ps[0:1, :], mybir.ActivationFunctionType.Sigmoid,
                    bias=bfc_sb[0:1, 0:1],
                )
                nc.sync.dma_start(y_d[c:c + 1, :], o[:])

            # ---- interleaved schedule ----
            # gemm0 chunk 0 upfront; later chunks stream one ahead of the
            # L0 recurrence inside the loop so the PE head stays short.
            gemm0(0)
            if nsc > 1:
                gemm0(1)
            for t in range(n_steps + delay):
                if t < n_steps and t % CH == 0 and t // CH + 2 < nsc:
                    gemm0(t // CH + 2)
                if t < n_steps:
                    rec_step(t, w0h_sb, xw0c, h0c, "z0")
                if t >= delay:
                    _xw1 = xw0c if os.environ.get("K_NO_GEMM1") else xw1c
                    rec_step(t - delay, w1h_sb, _xw1, h1c, "z1")
                if t < n_steps and (t + 1) % CH == 0:
                    if not os.environ.get("K_NO_GEMM1"):
                        gemm1((t + 1) // CH - 1)
                if t >= delay and (t - delay + 1) % CH == 0:
                    if not os.environ.get("K_NO_FC"):
                        fc((t - delay + 1) // CH - 1)
            if os.environ.get("K_NO_FC"):
                # still need the y output written once
                o0 = p_fco.tile([1, CH * BL], f32, tag="fco", name="fco_x")
                nc.vector.memset(o0[:], 0.0)
                for c in range(nsc):
                    nc.sync.dma_start(y_d[c:c + 1, :], o0[:])
            if debug_dumps:
                dc = int(os.environ.get("K_DBG_CHUNK", "0"))
                nc.sync.dma_start(dbg["dxw0"][:], xw0c[dc][:])
                nc.sync.dma_start(dbg["dh0"][:], h0c[dc][:])
                nc.sync.dma_start(dbg["dxw1"][:], xw1c[dc][:])
                nc.sync.dma_start(dbg["dh1"][:], h1c[dc][:])

    if split_waits:
        _split_multi_waits(nc)
    return nc


_cache = {}


def _get_nc(n_steps):
    if n_steps not in _cache:
        _cache[n_steps] = build_nc(n_steps)
    return _cache[n_steps]


def _wfc_host(W_fc):
    MF = 8
    w = np.zeros((KH, 128, MF), np.float32)
    w[:, :, 0] = W_fc.reshape(KH, 128)
    return np.ascontiguousarray(w.transpose(1, 0, 2).reshape(128, KH * MF)).astype(BF16)


def _prep_inputs(x, W_ih0, W_hh0, b_ih0, b_hh0, W_ih1, W_hh1, b_ih1, b_hh1,
                 W_fc, b_fc, n_steps=T):
    shared = {
        "w0i": np.ascontiguousarray(W_ih0.T).astype(BF16),
        "w0h": np.ascontiguousarray(W_hh0.T).astype(BF16),
        "w1i": np.ascontiguousarray(W_ih1.T).astype(BF16),
        "w1h": np.ascontiguousarray(W_hh1.T).astype(BF16),
        "wfc": _wfc_host(W_fc),
        "b0": np.ascontiguousarray((b_ih0 + b_hh0).reshape(KH, 128).T).astype(
            np.float32),
        "b1": np.ascontiguousarray((b_ih1 + b_hh1).reshape(KH, 128).T).astype(
            np.float32),
        "bfc": b_fc.reshape(1, 1).astype(np.float32),
    }
    in_maps = []
    for c in range(NCORES):
        xs = x[c * BL:(c + 1) * BL, :n_steps]          # [BL, n_steps, I]
        xt = np.ascontiguousarray(xs.transpose(2, 1, 0)).reshape(
            I, n_steps * BL)                            # col = t*BL + b
        in_maps.append({"xt": xt.astype(BF16), **shared})
    return in_maps


def _postprocess(results, n_steps=T):
    outs = []
    for c in range(NCORES):
        y = results[c]["y"].reshape(n_steps, BL)        # [t, b]
        outs.append(y.T)                                # [b, t]
    return np.concatenate(outs, axis=0)[:, :, None].astype(np.float32)


def kernel(x, W_ih0, W_hh0, b_ih0, b_hh0, W_ih1, W_hh1, b_ih1, b_hh1,
           W_fc, b_fc):
    x, W_ih0, W_hh0, b_ih0, b_hh0, W_ih1, W_hh1, b_ih1, b_hh1, W_fc, b_fc = [
        np.asarray(a, dtype=np.float32)
        for a in (x, W_ih0, W_hh0, b_ih0, b_hh0, W_ih1, W_hh1, b_ih1, b_hh1,
                  W_fc, b_fc)
    ]
    nc = _get_nc(T)
    in_maps = _prep_inputs(x, W_ih0, W_hh0, b_ih0, b_hh0, W_ih1, W_hh1,
                           b_ih1, b_hh1, W_fc, b_fc)
    res = run_bass_kernel_spmd(nc, in_maps, core_ids=list(range(NCORES)))
    return _postprocess(res.results)



# revision 2
# speedup vs baseline: 1.1870x; 1.1870x over previous
"""Layer-pipelined Trainium2 kernel for the 2-layer ReLU-RNN discriminator.

  B=64, T=512, I=256, H=512, O=1
  layer l: h_t = relu(x_t @ W_ih^T + b_ih + b_hh + h_{t-1} @ W_hh^T)
  out = sigmoid(h1 @ W_fc^T + b_fc)

Topology: 4 core-pairs (c, c+4). Core c in 0..3 runs LAYER 0 for sample
block [16c, 16c+16); core c+4 runs LAYER 1 for the same block. The pair
exchanges hidden-state chunks via a pair-wise AllGather (only the lo half
of the output is consumed = L0's h0; verified behavior on this stack).

All cores run the IDENTICAL program; roles differ only in data:
  - wg (input-projection weights over [x(2 kchunks) | recv(4 kchunks)]):
      L0: [W_ih0^T | 0], L1: [0 | W_ih1^T]
  - wh: W_hh0^T / W_hh1^T;  bias: b0 / b1
  - ebias ("early bias", used for iterations j < LAG): L0: b0 (its chunks
      are real from j=0), L1: -1e9 so relu clamps h to exactly 0 until the
      first real h0 chunk arrives -> correct zero initial state.
  - xt: L0: x^T of its block; L1: zeros.

Within a core: 16 samples as TWO 8-sample groups (A/B) advanced in
lockstep; each W_hh k,m-block is loaded once per step and feeds both
groups' matmuls (measured ~77ns per shared-weight block vs ~2x for
separate loads). Epilogue: relu on ACT (group A) and DVE (group B) in
parallel. PSUM chunk tiles are prefilled by the input GEMM (bias via K=1
matmuls), so the per-step epilogue is a single relu, no adds.
"""

import numpy as np
import ml_dtypes

import concourse.bass as bass
import concourse.mybir as mybir
from concourse.tile import TileContext
from concourse.bass_utils import run_bass_kernel_spmd
from concourse.alu_op_type import AluOpType

F16 = np.float16
NCORES = 8
B, T, I, H, O = 64, 512, 256, 512, 1
PAIRS = 4
NW = 16            # samples per core
GW = 8             # samples per group (2 groups)
CH = 16            # steps per chunk
SE = 8             # send (collective) every SE chunks
LAG = SE + 2       # pipeline lag (iterations) between send and consume
KH = H // 128      # 4
KX = I // 128      # 2
KG = KX + KH       # gemm contraction chunks
HC = KH * CH * GW  # h-tile cols per group = 512

_ctr = [0]


def _split_multi_waits(nc):
    """Walrus in this container rejects >1 sync-wait per instruction."""
    n_split = 0
    for f in nc.m.functions:
        for bb in f.blocks:
            out = []
            changed = False
            for inst in bb.instructions:
                si = inst.sync_info
                waits = list(si.on_wait) if si is not None and si.on_wait else []
                if len(waits) > 1:
                    changed = True
                    n_split += 1
                    for w in waits[:-1]:
                        _ctr[0] += 1
                        nop = mybir.InstNoOp(
                            name=f"waitnop-{_ctr[0]}", ins=[], outs=[]
                        )
                        nop.engine = inst.engine
                        nop.sync_info = mybir.SyncInfo(on_wait=[w], on_update=[])
                        out.append(nop)
                    inst.sync_info = mybir.SyncInfo(
                        on_wait=[waits[-1]],
                        on_update=list(si.on_update) if si.on_update else [],
                    )
                out.append(inst)
            if changed:
                bb.instructions = out
    return n_split


def build_nc(n_steps=T, debug_dumps=False, no_coll=False, kg=KG, no_fc=False,
             se=None):
    se = SE if se is None else se
    lag = se + 2
    nch = n_steps // CH
    niter = nch + lag
    nc = bass.Bass("TRN2", num_devices=NCORES)
    f32 = mybir.dt.float32
    bf = mybir.dt.float16

    xt_d = nc.dram_tensor("xt", [128, KX * 2 * n_steps * GW], bf,
                          kind="ExternalInput")
    wg_d = nc.dram_tensor("wg", [128, KG * H], bf, kind="ExternalInput")
    wh_d = nc.dram_tensor("wh", [128, KH * H], bf, kind="ExternalInput")
    bias_d = nc.dram_tensor("bias", [KH, 128], bf, kind="ExternalInput")
    ebias_d = nc.dram_tensor("ebias", [KH, 128], bf, kind="ExternalInput")
    wfc_d = nc.dram_tensor("wfc", [128, KH * 8], bf, kind="ExternalInput")
    ind_d = nc.dram_tensor("ind", [KH, KH * CH * GW], bf,
                           kind="ExternalInput")
    bfc_d = nc.dram_tensor("bfc", [1, 1], f32, kind="ExternalInput")
    y_d = nc.dram_tensor("y", [niter, CH * NW], f32, kind="ExternalOutput")
    if debug_dumps:
        dh_d = nc.dram_tensor("dh", [128, niter * 2 * HC], bf,
                              kind="ExternalOutput")

    with TileContext(nc) as tc:
        with (
            tc.tile_pool(name="wts", bufs=1) as p_w,
            tc.tile_pool(name="h", bufs=se + 2) as p_h,
            tc.tile_pool(name="recv", bufs=3 if se == 1 else 2) as p_recv,
            tc.tile_pool(name="y", bufs=2) as p_y,
            tc.tile_pool(name="ps", bufs=1, space="PSUM") as p_ps,
            tc.tile_pool(name="fc", bufs=2, space="PSUM") as p_fc,
            tc.tile_pool(name="dram", bufs=3, space="DRAM") as p_dram,
        ):
            # ---- load inputs ----
            xt_sb = p_w.tile([128, KX * 2 * n_steps * GW], bf, tag="xt")
            nc.sync.dma_start(xt_sb[:], xt_d[:])
            wg_sb = p_w.tile([128, KG * H], bf, tag="wg")
            nc.sync.dma_start(wg_sb[:], wg_d[:])
            wh_sb = p_w.tile([128, KH * H], bf, tag="wh")
            nc.sync.dma_start(wh_sb[:], wh_d[:])
            bias_sb = p_w.tile([KH, 128], bf, tag="bias")
            nc.sync.dma_start(bias_sb[:], bias_d[:])
            ebias_sb = p_w.tile([KH, 128], bf, tag="ebias")
            nc.sync.dma_start(ebias_sb[:], ebias_d[:])
            wfc_sb = p_w.tile([128, KH * 8], bf, tag="wfc")
            nc.sync.dma_start(wfc_sb[:], wfc_d[:])
            bfc_sb = p_w.tile([1, 1], f32, tag="bfc")
            nc.sync.dma_start(bfc_sb[:], bfc_d[:])
            ind_sb = p_w.tile([KH, KH * CH * GW], bf, tag="ind")
            nc.sync.dma_start(ind_sb[:], ind_d[:])
            recv_zero = p_w.tile([128, 2 * HC], bf, tag="rz")
            nc.vector.memset(recv_zero[:], 0.0)

            recv_tiles = []
            ps_tiles = {}   # j -> (psA, psB)
            h_tiles = {}    # (j, g) -> tile

            def mslice(ps, m, r=None):
                if r is None:
                    return ps[:, m * CH * GW:(m + 1) * CH * GW]
                return ps[:, m * CH * GW + r * GW:m * CH * GW + (r + 1) * GW]

            # ---- input GEMM: prefill psum for chunk j (both groups) ----
            # Returns a list of thunks (one matmul each) so the caller can
            # interleave them into the chain steps as PE filler work that
            # absorbs the relu-epilogue round-trip latency.
            def gemm_thunks(j):
                if j < lag:
                    rsb, q = recv_zero, 0
                else:
                    sidx, q = divmod(j - lag, se)
                    rsb = recv_tiles[sidx]
                bsb = ebias_sb if j < lag else bias_sb
                ps = [p_ps.tile([128, KH * CH * GW], f32, tag=f"ps{g}{j % 2}",
                                name=f"ps{g}_{j}") for g in range(2)]
                ps_tiles[j] = ps
                thunks = []
                for g in range(2):
                    thunks.append(lambda g=g: nc.tensor.matmul(
                        ps[g][:], bsb[:], ind_sb[:],
                        start=True, stop=False,
                    ))
                for k in range(kg):
                    for m in range(KH):
                        w = wg_sb[:, k * H + m * 128:k * H + (m + 1) * 128]
                        for g in range(2):
                            if k < KX:
                                off = ((k * 2 + g) * n_steps + (j % nch) * CH) * GW
                                mov = xt_sb[:, off:off + CH * GW]
                            else:
                                off = ((q * 2 + g) * HC
                                       + (k - KX) * CH * GW)
                                mov = rsb[:, off:off + CH * GW]
                            thunks.append(
                                lambda g=g, m=m, w=w, mov=mov, k=k, ps=ps:
                                nc.tensor.matmul(
                                    mslice(ps[g], m), w, mov,
                                    start=False, stop=(k == kg - 1),
                                ))
                return thunks

            # ---- one chain step (both groups) ----
            # PE order: for k: for m: [mm_A, mm_B] — same-weight mms
            # adjacent so the stationary load is paid once per (k, m).
            def step(j, r):
                ps = ps_tiles[j]
                if not (j == 0 and r == 0):
                    rp = r - 1 if r > 0 else CH - 1
                    for k in range(KH):
                        ksrc = []
                        for g in range(2):
                            hsrc = (h_tiles[(j, g)] if r > 0
                                    else h_tiles[(j - 1, g)])
                            ksrc.append(
                                hsrc[:, k * CH * GW + rp * GW:
                                     k * CH * GW + (rp + 1) * GW])
                        for m in range(KH):
                            w = wh_sb[:, k * H + m * 128:k * H + (m + 1) * 128]
                            for g in range(2):
                                nc.tensor.matmul(
                                    mslice(ps[g], m, r),
                                    w,
                                    ksrc[g],
                                    start=False, stop=(k == KH - 1),
                                )
                # epilogue: relu psum -> h (A on ACT, B on DVE)
                for g in range(2):
                    h3 = h_tiles[(j, g)][:].rearrange(
                        "p (k x) -> p k x", k=KH)[:, :, r * GW:(r + 1) * GW]
                    p3 = ps[g][:].rearrange(
                        "p (m x) -> p m x", m=KH)[:, :, r * GW:(r + 1) * GW]
                    if g == 0:
                        nc.scalar.activation(
                            h3, p3, mybir.ActivationFunctionType.Relu)
                    else:
                        nc.vector.tensor_scalar_max(h3, p3, 0.0)

            # NOTE on step() weight sharing: the PE stream per (k) is
            # [mm(g=0,m=0..3), mm(g=1,m=0..3)] — reordered below to pair
            # same-weight mms adjacently.

            def fc(j):
                ps = p_fc.tile([8, 2 * CH * GW], f32, tag=f"fc{j % 2}",
                               name=f"fc{j}")
                for g in range(2):
                    for k in range(KH):
                        nc.tensor.matmul(
                            ps[:, g * CH * GW:(g + 1) * CH * GW],
                            wfc_sb[:, k * 8:(k + 1) * 8],
                            h_tiles[(j, g)][:, k * CH * GW:(k + 1) * CH * GW],
                            start=(k == 0), stop=(k == KH - 1),
                        )
                ysb = p_y.tile([1, 2 * CH * GW], f32, tag="y", name=f"y{j}")
                nc.scalar.activation(
                    ysb[:], ps[0:1, :], mybir.ActivationFunctionType.Sigmoid,
                    bias=bfc_sb[0:1, 0:1],
                )
                nc.sync.dma_start(y_d[j:j + 1, :], ysb[:])

            def send(j):
                if (j + 1) % se != 0:
                    return
                sidx = j // se
                bi = p_dram.tile([128, se * 2 * HC], bf, tag="bi",
                                 name=f"bi{sidx}")
                for qq in range(se):
                    jq = j - se + 1 + qq
                    for g in range(2):
                        nc.gpsimd.dma_start(
                            bi[:, (qq * 2 + g) * HC:(qq * 2 + g + 1) * HC],
                            h_tiles[(jq, g)][:])
                rs = p_recv.tile([128, se * 2 * HC], bf, tag="recv",
                                 name=f"rs{sidx}")
                if no_coll:
                    nc.gpsimd.dma_start(rs[:], bi[:])
                else:
                    bo = p_dram.tile([256, se * 2 * HC], bf, tag="bo",
                                     name=f"bo{sidx}")
                    nc.gpsimd.collective_compute(
                        "AllGather",
                        mybir.AluOpType.bypass,
                        replica_groups=[[0, 4], [1, 5], [2, 6], [3, 7]],
                        ins=[bi.opt()],
                        outs=[bo.opt()],
                    )
                    nc.gpsimd.dma_start(rs[:], bo[0:128, :])
                recv_tiles.append(rs)

            # ---- main loop ----
            for t in gemm_thunks(0):
                t()
            for j in range(niter):
                for g in range(2):
                    h_tiles[(j, g)] = p_h.tile([128, HC], bf, tag=f"h{g}",
                                               name=f"h{g}_{j}")
                pend = gemm_thunks(j + 1) if j + 1 < niter else []
                per = -(-len(pend) // CH) if pend else 0
                for r in range(CH):
                    step(j, r)
                    for t in pend[r * per:(r + 1) * per]:
                        t()
                if not no_fc:
                    fc(j)
                send(j)
                if debug_dumps:
                    for g in range(2):
                        nc.sync.dma_start(
                            dh_d[:, (j * 2 + g) * HC:(j * 2 + g + 1) * HC],
                            h_tiles[(j, g)][:])

    _split_multi_waits(nc)
    return nc


_cache = {}


def _get_nc(n_steps):
    if n_steps not in _cache:
        _cache[n_steps] = build_nc(n_steps)
    return _cache[n_steps]


def _prep_inputs(x, W_ih0, W_hh0, b_ih0, b_hh0, W_ih1, W_hh1, b_ih1, b_hh1,
                 W_fc, b_fc, n_steps=T):
    nch = n_steps // CH

    def wT(w):  # [out, in] -> [in, out] contiguous bf16
        return np.ascontiguousarray(w.T).astype(F16)

    wg0 = np.zeros((KG * 128, H), np.float32)
    wg0[0:I, :] = W_ih0.T
    wg1 = np.zeros((KG * 128, H), np.float32)
    wg1[KX * 128:KX * 128 + H, :] = W_ih1.T

    def wg_pack(wg):  # [KG*128, H] -> [128, KG*H]
        return np.ascontiguousarray(
            wg.reshape(KG, 128, H).transpose(1, 0, 2).reshape(128, KG * H)
        ).astype(F16)

    def wh_pack(whh):  # W_hh [H,H] -> lhsT chunks [128, KH*H]
        t = whh.T.reshape(KH, 128, H).transpose(1, 0, 2)
        return np.ascontiguousarray(t.reshape(128, KH * H)).astype(F16)

    wfc = np.zeros((KH, 128, 8), np.float32)
    wfc[:, :, 0] = W_fc.reshape(KH, 128)
    wfc = np.ascontiguousarray(
        wfc.transpose(1, 0, 2).reshape(128, KH * 8)).astype(F16)

    b0 = (b_ih0 + b_hh0).reshape(KH, 128).astype(F16)
    b1 = (b_ih1 + b_hh1).reshape(KH, 128).astype(F16)
    neg = np.full((KH, 128), -60000.0, F16)
    ind = np.zeros((KH, KH * CH * GW), np.float32)
    for c in range(KH):
        ind[c, c * CH * GW:(c + 1) * CH * GW] = 1.0
    ind = ind.astype(F16)
    bfc = b_fc.reshape(1, 1).astype(np.float32)

    in_maps = []
    for c in range(NCORES):
        p = c % PAIRS
        role = c // PAIRS
        if role == 0:
            xs = x[p * NW:(p + 1) * NW, :n_steps]        # [16, t, I]
            # layout [kx][g][t][gw]: value x[g*8+b, t, kx*128+i]
            xt = xs.reshape(2, GW, n_steps, KX, 128)     # [g][b][t][kx][i]
            xt = xt.transpose(4, 3, 0, 2, 1)             # [i][kx][g][t][b]
            xt = np.ascontiguousarray(
                xt.reshape(128, KX * 2 * n_steps * GW)).astype(F16)
        else:
            xt = np.zeros((128, KX * 2 * n_steps * GW), F16)
        in_maps.append({
            "xt": xt,
            "wg": wg_pack(wg0 if role == 0 else wg1),
            "wh": wh_pack(W_hh0 if role == 0 else W_hh1),
            "bias": b0 if role == 0 else b1,
            "ebias": b0 if role == 0 else neg,
            "wfc": wfc,
            "bfc": bfc,
            "ind": ind,
        })
    return in_maps


def _postprocess(results, n_steps=T, se=None):
    lag = (SE if se is None else se) + 2
    nch = n_steps // CH
    out = np.zeros((B, n_steps, 1), np.float32)
    for p in range(PAIRS):
        y = results[PAIRS + p]["y"]                      # [niter, CH*NW]
        y = y[lag:lag + nch].reshape(nch, 2, CH, GW)     # [i][g][r][b]
        for g in range(2):
            blk = y[:, g, :, :].transpose(2, 0, 1).reshape(GW, n_steps)
            out[p * NW + g * GW:p * NW + (g + 1) * GW, :, 0] = blk
    return out


def kernel(x, W_ih0, W_hh0, b_ih0, b_hh0, W_ih1, W_hh1, b_ih1, b_hh1,
           W_fc, b_fc):
    args = [np.asarray(a, dtype=np.float32)
            for a in (x, W_ih0, W_hh0, b_ih0, b_hh0, W_ih1, W_hh1, b_ih1,
                      b_hh1, W_fc, b_fc)]
    nc = _get_nc(T)
    in_maps = _prep_inputs(*args)
    res = run_bass_kernel_spmd(nc, in_maps, core_ids=list(range(NCORES)))
    return _postprocess(res.results)
